# revision 1
# baseline (speedup 1.0000x reference)
"""Distributed BatchSpectralLoss kernel for Trainium2 (8 NeuronCores).

Computes sum of top-k squared singular values of x (= top-k eigenvalues of
the Gram matrix G = x^T x) for x of shape (8192, 4096), k small (k=1).

Algorithm (all device matmuls in bf16 with fp32 PSUM accumulation):
  1. Host: estimate lambda_1 cheaply (block power iteration), scale x by
     1/sqrt(C) and cast to bf16, so the device Gram directly yields A0 = G/C.
  2. Device, sharded across 8 cores (core r owns rows r*512..(r+1)*512 of
     every produced matrix; AllGather of row slices re-assembles plain
     row-major full matrices):
       - g_slice = xcols_r^T @ x          (A0 row-slice)        + AllGather
       - p1 squarings: a_slice = a_slice^T(kxm) @ a_full        + AllGather
         (A_{j+1} = A_j^2; symmetry lets the core's own row-slice, read
          transposed, serve as the lhsT column-slice)
       - block power phase (hand-written, persistent SBUF, `chains`
         independent chains interleaved so one chain's matmuls hide the
         other's AllGather): y_slice = A_p1[rows_r,:] @ y_full, AllGather
       - w = one more application per chain (no AllGather)
       - S1[ci,cj] = Y_ci^T W_cj, S0[ci,cj] = Y_ci^T Y_cj  (partial over the
         core's rows; host sums partials over cores)
  3. Host: generalized Ritz eigenvalues theta_i of (S1, S0) approximate
     lambda_i(A_p1); lambda_i(G) = C * theta_i^(1/2^p1); answer = sum top k.

The 2^-p1 root compresses block-phase and late rounding errors by 2^p1; the
p1 squarings make s block steps act like a degree s*2^p1 polynomial filter.
"""

import numpy as np
import ml_dtypes

N_CORES = 8
M_ROWS = 8192
N_DIM = 4096
P1 = 2
S_STEPS = 8
B_BLOCK = 128
CHAINS = 2

_NC_CACHE: dict = {}


def _est_scale(x_np, iters=15, blk=4):
    """Host block-power-iteration estimate of lambda_1(x^T x).

    Only used to pick the static normalization C; range safety needs C within
    ~±20% of lambda_1, which ~15 block iterations comfortably deliver for any
    PSD spectrum. Returns 1.10 * max Rayleigh quotient (mild overshoot keeps
    the squaring chain's magnitudes shrinking rather than growing).
    """
    rng = np.random.default_rng(0)
    v = rng.standard_normal((x_np.shape[1], blk)).astype(np.float32)
    v /= np.linalg.norm(v, axis=0, keepdims=True)
    for _ in range(iters):
        w = x_np.T @ (x_np @ v)
        v = w / np.linalg.norm(w, axis=0, keepdims=True)
    x64 = x_np.astype(np.float64)
    v64 = v.astype(np.float64)
    v64 /= np.linalg.norm(v64, axis=0, keepdims=True)
    ray = ((x64 @ v64) ** 2).sum(axis=0)
    return 1.10 * float(ray.max())


def _build_nc(m_rows, n_dim, b, p1, s, n_cores, chains, enable_asserts=False):
    import concourse.mybir as mybir
    import concourse.tile as tile
    from concourse import bacc
    import concourse.kernels.tile_matmul as tm
    from contextlib import ExitStack

    orig_comp = tm.composable_matmul_tile_kernel

    def comp_psum2(*a, **kw):
        kw.setdefault("psum_n_bufs", 2)
        return orig_comp(*a, **kw)

    def matmul_tile_kernel(*a, **kw):
        tm.composable_matmul_tile_kernel = comp_psum2
        try:
            return tm.matmul_tile_kernel(*a, **kw)
        finally:
            tm.composable_matmul_tile_kernel = orig_comp

    P = 128
    sl = n_dim // n_cores  # 512 rows per core
    msub = sl // P         # 4
    kpo = n_dim // P       # 32
    bf = mybir.dt.bfloat16
    f32 = mybir.dt.float32
    nc = bacc.Bacc(
        "TRN2",
        target_bir_lowering=False,
        debug=False,
        enable_asserts=enable_asserts,
        num_devices=n_cores,
    )

    xb = nc.dram_tensor("xb", [m_rows, n_dim], bf, kind="ExternalInput")
    xcols = nc.dram_tensor("xcols", [m_rows, sl], bf, kind="ExternalInput")
    omegas = [
        nc.dram_tensor(f"omega{c}", [n_dim, b], bf, kind="ExternalInput")
        for c in range(chains)
    ]
    nb = chains * b
    s1r = nc.dram_tensor("s1r", [nb, nb], f32, kind="ExternalOutput")
    s0r = nc.dram_tensor("s0r", [nb, nb], f32, kind="ExternalOutput")

    g_slice = nc.dram_tensor("g_slice", [sl, n_dim], bf)
    a_full = [
        nc.dram_tensor(f"a_full_{j}", [n_dim, n_dim], bf, addr_space="Shared")
        for j in range(p1)
    ]
    a_slice = [nc.dram_tensor(f"a_slice_{j}", [sl, n_dim], bf) for j in range(p1)]
    y_slice = [
        [nc.dram_tensor(f"y_slice_{c}_{t}", [sl, b], bf) for t in range(s)]
        for c in range(chains)
    ]
    y_full = [
        [
            nc.dram_tensor(f"y_full_{c}_{t}", [n_dim, b], bf, addr_space="Shared")
            for t in range(s)
        ]
        for c in range(chains)
    ]

    rg = [list(range(n_cores))]

    def dve_evict(nc_, psum, sbuf):
        nc_.vector.tensor_copy(out=sbuf, in_=psum)

    def ag(inp, outp):
        nc.gpsimd.collective_compute(
            "AllGather",
            mybir.AluOpType.bypass,
            replica_groups=rg,
            ins=[inp.ap().opt()],
            outs=[outp.ap().opt()],
        )

    with tile.TileContext(nc) as tc:
        # ---- Gram row-slice: G[rows_r, :] = xcols^T @ x ----
        matmul_tile_kernel(
            tc, kxm_ap=xcols.ap(), kxn_ap=xb.ap(), mxn_ap=g_slice.ap(),
            psum_evict_fn=dve_evict,
        )
        prev_s = g_slice
        if p1 > 0:
            ag(g_slice, a_full[0])
            prev_f = a_full[0]
            for j in range(p1):
                matmul_tile_kernel(
                    tc,
                    kxm_ap=prev_s.ap(),
                    kxn_ap=prev_f.ap(),
                    mxn_ap=a_slice[j].ap(),
                    transpose_kxm=True,
                    psum_evict_fn=dve_evict,
                )
                prev_s = a_slice[j]
                if j + 1 < p1:
                    ag(prev_s, a_full[j + 1])
                    prev_f = a_full[j + 1]

        ak = prev_s  # [sl, n_dim] row-slice of A_{p1} (= its column-slice^T)

        # ---- hand-written block power phase ----
        with ExitStack() as ctx:
            cpool = ctx.enter_context(tc.tile_pool(name="blk_const", bufs=1))
            ypool = ctx.enter_context(tc.tile_pool(name="blk_y", bufs=3))
            opool = ctx.enter_context(tc.tile_pool(name="blk_out", bufs=3))
            # PSUM budget: 8 banks total = chains*3 (block) + 2 (S-forms)
            pspool = ctx.enter_context(
                tc.tile_pool(name="blk_psum", bufs=3, space="PSUM")
            )

            # kc[pi, po, f] = A_p1[row f (slice-local), col po*128+pi]
            kc = cpool.tile([P, kpo, sl], bf, tag="kc")
            ak3 = ak.ap().rearrange("f (po pi) -> f po pi", pi=P)
            for kt in range(kpo // 4):
                nc.sync.dma_start_transpose(
                    kc[:, kt * 4 : (kt + 1) * 4, :], ak3[:, kt * 4 : (kt + 1) * 4, :]
                )

            y_cur = []
            for c in range(chains):
                ysb = ypool.tile([P, kpo, b], bf, tag=f"yf{c}")
                nc.sync.dma_start(
                    ysb[:], omegas[c].ap().rearrange("(po pi) b -> pi po b", pi=P)
                )
                y_cur.append(ysb)

            def chain_step(c, dst_sb):
                """dst_sb[pi, mo, :] = (A_p1 @ Y_c)[rows_r] in bf16."""
                for mo in range(msub):
                    ps = pspool.tile([P, b], f32, tag=f"ps{c}")
                    for ko in range(kpo):
                        nc.tensor.matmul(
                            ps[:],
                            kc[:, ko, mo * P : (mo + 1) * P],
                            y_cur[c][:, ko, :],
                            start=(ko == 0),
                            stop=(ko == kpo - 1),
                        )
                    nc.vector.tensor_copy(dst_sb[:, mo, :], ps[:])

            ys_final = [None] * chains
            for t in range(s):
                for c in range(chains):
                    last = t == s - 1
                    if last:
                        out_sb = cpool.tile([P, msub, b], bf, tag=f"ysfin{c}")
                        ys_final[c] = out_sb
                    else:
                        out_sb = opool.tile([P, msub, b], bf, tag=f"yo{c}")
                    chain_step(c, out_sb)
                    nc.sync.dma_start(
                        y_slice[c][t].ap().rearrange("(mo pi) b -> pi mo b", pi=P),
                        out_sb[:],
                    )
                    ag(y_slice[c][t], y_full[c][t])
                    ysb = ypool.tile([P, kpo, b], bf, tag=f"yf{c}")
                    nc.sync.dma_start(
                        ysb[:],
                        y_full[c][t].ap().rearrange("(po pi) b -> pi po b", pi=P),
                    )
                    y_cur[c] = ysb

            # one more application per chain: W_c = A_p1 @ Y_c (slice, no AG)
            w_sb = []
            for c in range(chains):
                wsb = cpool.tile([P, msub, b], bf, tag=f"w{c}")
                chain_step(c, wsb)
                w_sb.append(wsb)

            # ---- S forms: S1[ci,cj] = Y_ci^T W_cj, S0[ci,cj] = Y_ci^T Y_cj ----
            spool = ctx.enter_context(tc.tile_pool(name="s_out", bufs=2))
            pspool2 = ctx.enter_context(
                tc.tile_pool(name="s_psum", bufs=2, space="PSUM")
            )
            for ci in range(chains):
                for cj in range(chains):
                    for which, rhs_sb, out_t in (
                        ("s1", w_sb[cj], s1r),
                        ("s0", ys_final[cj], s0r),
                    ):
                        ps = pspool2.tile([b, b], f32, tag="ps_s")
                        for ko in range(msub):
                            nc.tensor.matmul(
                                ps[:],
                                ys_final[ci][:, ko, :],
                                rhs_sb[:, ko, :],
                                start=(ko == 0),
                                stop=(ko == msub - 1),
                            )
                        osb = spool.tile([b, b], f32, tag="osb")
                        nc.vector.tensor_copy(osb[:], ps[:])
                        nc.sync.dma_start(
                            out_t.ap()[ci * b : (ci + 1) * b, cj * b : (cj + 1) * b],
                            osb[:],
                        )

    nc.compile()
    return nc


def _get_nc(cfg):
    if cfg not in _NC_CACHE:
        _NC_CACHE[cfg] = _build_nc(*cfg)
    return _NC_CACHE[cfg]


def _ritz_topk(S1, S0, k):
    """Top-k generalized eigenvalues of (S1, S0), f64, rank-guarded."""
    S1 = 0.5 * (S1 + S1.T)
    S0 = 0.5 * (S0 + S0.T)
    d = np.sqrt(np.clip(np.diag(S0), 0, None))
    d = np.where(d > 0, d, 1.0)
    dn = 1.0 / d
    S0n = S0 * dn[:, None] * dn[None, :]
    S1n = S1 * dn[:, None] * dn[None, :]
    w0, v0 = np.linalg.eigh(S0n)
    keep = w0 > (w0.max() * 1e-4)
    v = v0[:, keep] / np.sqrt(w0[keep])[None, :]
    m = v.T @ S1n @ v
    m = 0.5 * (m + m.T)
    ev = np.linalg.eigvalsh(m)
    ev = np.clip(ev, 0.0, None)
    return np.sort(ev)[::-1][:k]


def _host_solve(results, k, p1, c_scale):
    S1 = np.zeros_like(results[0]["s1r"], dtype=np.float64)
    S0 = np.zeros_like(results[0]["s0r"], dtype=np.float64)
    for r in results:
        S1 += r["s1r"].astype(np.float64)
        S0 += r["s0r"].astype(np.float64)
    thetas = _ritz_topk(S1, S0, k)
    lams = c_scale * np.power(np.clip(thetas, 1e-300, None), 1.0 / (1 << p1))
    return float(np.sum(lams))


def _make_inputs(x_np, b, n_cores, c_scale, chains):
    n_dim = x_np.shape[1]
    sl = n_dim // n_cores
    bf = ml_dtypes.bfloat16
    xs = (x_np.astype(np.float64) / np.sqrt(c_scale)).astype(np.float32)
    xb = np.ascontiguousarray(xs.astype(bf))
    rng = np.random.default_rng(12345)
    omegas = [
        np.ascontiguousarray(
            rng.standard_normal((n_dim, b)).astype(np.float32).astype(bf)
        )
        for _ in range(chains)
    ]
    in_maps = []
    for r in range(n_cores):
        m = {
            "xb": xb,
            "xcols": np.ascontiguousarray(xb[:, r * sl : (r + 1) * sl]),
        }
        for c in range(chains):
            m[f"omega{c}"] = omegas[c]
        in_maps.append(m)
    return in_maps


def _host_fallback(x_np, k_int):
    """Correct-but-slow host path, used only if the device result is bad."""
    import scipy.linalg

    g = x_np.astype(np.float64).T @ x_np.astype(np.float64)
    n = g.shape[0]
    ev = scipy.linalg.eigh(g, eigvals_only=True, subset_by_index=[n - k_int, n - 1])
    return float(np.sum(ev))


def kernel(x, k):
    from concourse.bass_utils import run_bass_kernel_spmd

    x_np = np.asarray(x, dtype=np.float32)
    k_int = int(np.asarray(k))
    if k_int <= 0:
        return np.asarray(0.0, dtype=np.float32)

    try:
        c_scale = _est_scale(x_np)
        cfg = (M_ROWS, N_DIM, B_BLOCK, P1, S_STEPS, N_CORES, CHAINS)
        nc = _get_nc(cfg)
        in_maps = _make_inputs(x_np, B_BLOCK, N_CORES, c_scale, CHAINS)
        res = run_bass_kernel_spmd(nc, in_maps, core_ids=list(range(N_CORES)))
        val = _host_solve(res.results, k_int, P1, c_scale)
        if not np.isfinite(val) or val <= 0:
            raise FloatingPointError(f"bad device result {val}")
    except Exception:
        val = _host_fallback(x_np, k_int)
    return np.asarray(val, dtype=np.float32)



# revision 2
# speedup vs baseline: 2.4867x; 2.4867x over previous
"""Distributed BatchSpectralLoss kernel for Trainium2 (8 NeuronCores).

Computes sum of top-k squared singular values of x (= top-k eigenvalues of
the Gram matrix G = x^T x) for x of shape (8192, 4096), k small (k=1).

Algorithm — implicit block Krylov on x (G is never formed):
  Host: scale x by 1/sqrt(C) with C = 3*||x||_F^2/N so lamhat_1 = O(1) in
  bf16, and draw `chains` random start blocks Omega [4096, b].
  Device, per core r (bf16 matmuls, fp32 PSUM):
    SBUF-resident x slices: xrT = x[rows_r,:]^T (lhsT for U = x_r @ Y) and
    xc = x[:, cols_r] (lhsT for the Y-update), rows_r = 1024 rows,
    cols_r = 512 cols.
    Per application t (q per chain, chains interleaved so one chain's
    matmuls hide the other's collectives):
      U_r = x[rows_r,:] @ Y_t          [1024, b]   (local rows)
      AllGather U slices -> U_full     [8192, b]
      Y_{t+1}[cols_r] = xc^T @ U_full  [512, b]    (local-complete, no AR)
      AllGather Y slices -> Y_full (skipped for the last level)
    Basis Gram over the core's 512 rows: P[a,bb] = Y_a^T Y_bb for all
    stored levels (upper triangle); host sums partials over cores.
  Host: S0 = P[basis, basis], S1 = P[basis, basis+1] (since
  Y_{t+1} = Ghat Y_t, so Y_i^T Ghat Y_j = Y_i^T Y_{j+1}); rank-guarded
  generalized Ritz values theta of (S1, S0); lambda = C * theta;
  answer = sum of top k.
"""

import numpy as np
import ml_dtypes

N_CORES = 8
M_ROWS = 8192
N_DIM = 4096
B_BLOCK = 128
Q_APPS = 5
CHAINS = 2
CLIP_TH = 1e-5

_NC_CACHE: dict = {}


def _build_nc(m_rows, n_dim, b, q, n_cores, chains, enable_asserts=False):
    import concourse.mybir as mybir
    import concourse.tile as tile
    from concourse import bacc
    from contextlib import ExitStack

    P = 128
    mloc = m_rows // n_cores   # 1024 rows of x per core
    nloc = n_dim // n_cores    # 512 cols of x per core
    ko_u = n_dim // P          # 32 k-tiles for U-matmul
    ko_y = m_rows // P         # 64 k-tiles for Y-matmul
    mo_u = mloc // P           # 8 output tiles of U slice
    mo_y = nloc // P           # 4 output tiles of Y slice
    nlev = q + 1               # stored levels 0..q per chain
    nblk = chains * nlev
    bf = mybir.dt.bfloat16
    f32 = mybir.dt.float32

    nc = bacc.Bacc(
        "TRN2",
        target_bir_lowering=False,
        debug=False,
        enable_asserts=enable_asserts,
        num_devices=n_cores,
    )

    xrT = nc.dram_tensor("xrT", [n_dim, mloc], bf, kind="ExternalInput")
    xc = nc.dram_tensor("xc", [m_rows, nloc], bf, kind="ExternalInput")
    omegas = [
        nc.dram_tensor(f"omega{c}", [n_dim, b], bf, kind="ExternalInput")
        for c in range(chains)
    ]
    omsl = [
        nc.dram_tensor(f"omsl{c}", [nloc, b], bf, kind="ExternalInput")
        for c in range(chains)
    ]
    p_out = nc.dram_tensor("p_out", [nblk * b, nblk * b], f32, kind="ExternalOutput")

    u_sl_d = [[nc.dram_tensor(f"usl_{c}_{t}", [mloc, b], bf) for t in range(q)]
              for c in range(chains)]
    u_fl_d = [[nc.dram_tensor(f"ufl_{c}_{t}", [m_rows, b], bf, addr_space="Shared")
               for t in range(q)] for c in range(chains)]
    y_sl_d = [[nc.dram_tensor(f"ysl_{c}_{t}", [nloc, b], bf) for t in range(q)]
              for c in range(chains)]
    y_fl_d = [[nc.dram_tensor(f"yfl_{c}_{t}", [n_dim, b], bf, addr_space="Shared")
               for t in range(q)] for c in range(chains)]

    rg = [list(range(n_cores))]

    def ag(inp, outp):
        nc.gpsimd.collective_compute(
            "AllGather",
            mybir.AluOpType.bypass,
            replica_groups=rg,
            ins=[inp.ap().opt()],
            outs=[outp.ap().opt()],
        )

    with tile.TileContext(nc) as tc, ExitStack() as ctx:
        xpool = ctx.enter_context(tc.tile_pool(name="xin", bufs=1))
        ypool = ctx.enter_context(tc.tile_pool(name="yfull", bufs=1))
        upool = ctx.enter_context(tc.tile_pool(name="ufull", bufs=1))
        slpool = ctx.enter_context(tc.tile_pool(name="slices", bufs=1))
        opool = ctx.enter_context(tc.tile_pool(name="evict", bufs=1))
        ppool = ctx.enter_context(tc.tile_pool(name="pout", bufs=3))
        # PSUM: 8 banks = chains*3 (application phase) + 2 (P-forms)
        pspool = ctx.enter_context(tc.tile_pool(name="ps", bufs=3, space="PSUM"))
        pspool2 = ctx.enter_context(tc.tile_pool(name="psp", bufs=2, space="PSUM"))

        xrT_sb = xpool.tile([P, ko_u, mloc], bf, tag="xrT")
        nc.sync.dma_start(xrT_sb[:], xrT.ap().rearrange("(ko pi) m -> pi ko m", pi=P))
        xc_sb = xpool.tile([P, ko_y, nloc], bf, tag="xc")
        nc.sync.dma_start(xc_sb[:], xc.ap().rearrange("(ko pi) m -> pi ko m", pi=P))

        ysl = {}
        ycur = {}
        for c in range(chains):
            yf = ypool.tile([P, ko_u, b], bf, tag=f"yf{c}")
            nc.sync.dma_start(yf[:], omegas[c].ap().rearrange("(ko pi) b -> pi ko b", pi=P))
            ycur[c] = yf
            s = slpool.tile([P, mo_y, b], bf, tag=f"ysl{c}_0")
            nc.sync.dma_start(s[:], omsl[c].ap().rearrange("(mo pi) b -> pi mo b", pi=P))
            ysl[(c, 0)] = s

        for t in range(q):
            for c in range(chains):
                # U slice = x[rows_r, :] @ Y_t
                usb = opool.tile([P, mo_u, b], bf, tag=f"u{c}")
                for mo in range(mo_u):
                    ps = pspool.tile([P, b], f32, tag=f"ps{c}")
                    for ko in range(ko_u):
                        nc.tensor.matmul(
                            ps[:],
                            xrT_sb[:, ko, mo * P:(mo + 1) * P],
                            ycur[c][:, ko, :],
                            start=(ko == 0),
                            stop=(ko == ko_u - 1),
                        )
                    nc.vector.tensor_copy(usb[:, mo, :], ps[:])
                nc.sync.dma_start(
                    u_sl_d[c][t].ap().rearrange("(mo pi) b -> pi mo b", pi=P), usb[:]
                )
                ag(u_sl_d[c][t], u_fl_d[c][t])
                ufs = upool.tile([P, ko_y, b], bf, tag=f"uf{c}")
                nc.sync.dma_start(
                    ufs[:], u_fl_d[c][t].ap().rearrange("(ko pi) b -> pi ko b", pi=P)
                )
                # Y_{t+1} slice = x[:, cols_r]^T @ U_full (local-complete)
                ss = slpool.tile([P, mo_y, b], bf, tag=f"ysl{c}_{t + 1}")
                for mo in range(mo_y):
                    ps = pspool.tile([P, b], f32, tag=f"ps{c}")
                    for ko in range(ko_y):
                        nc.tensor.matmul(
                            ps[:],
                            xc_sb[:, ko, mo * P:(mo + 1) * P],
                            ufs[:, ko, :],
                            start=(ko == 0),
                            stop=(ko == ko_y - 1),
                        )
                    nc.vector.tensor_copy(ss[:, mo, :], ps[:])
                ysl[(c, t + 1)] = ss
                if t + 1 < q:
                    nc.sync.dma_start(
                        y_sl_d[c][t + 1].ap().rearrange("(mo pi) b -> pi mo b", pi=P),
                        ss[:],
                    )
                    ag(y_sl_d[c][t + 1], y_fl_d[c][t + 1])
                    yf = ypool.tile([P, ko_u, b], bf, tag=f"yf{c}")
                    nc.sync.dma_start(
                        yf[:], y_fl_d[c][t + 1].ap().rearrange("(ko pi) b -> pi ko b", pi=P)
                    )
                    ycur[c] = yf

        # Basis Gram over the core's 512 rows: upper-triangle blocks only.
        blocks = [(c, t) for c in range(chains) for t in range(nlev)]
        for a in range(nblk):
            for bb in range(a, nblk):
                ps = pspool2.tile([b, b], f32, tag="psp")
                ta = ysl[blocks[a]]
                tb = ysl[blocks[bb]]
                for ko in range(mo_y):
                    nc.tensor.matmul(
                        ps[:],
                        ta[:, ko, :],
                        tb[:, ko, :],
                        start=(ko == 0),
                        stop=(ko == mo_y - 1),
                    )
                ob = ppool.tile([b, b], f32, tag="ob")
                nc.vector.tensor_copy(ob[:], ps[:])
                nc.sync.dma_start(
                    p_out.ap()[a * b:(a + 1) * b, bb * b:(bb + 1) * b], ob[:]
                )

    nc.compile()
    return nc


def _get_nc(cfg):
    if cfg not in _NC_CACHE:
        _NC_CACHE[cfg] = _build_nc(*cfg)
    return _NC_CACHE[cfg]


def _ritz_topk(S1, S0, k):
    """Top-k generalized eigenvalues of (S1, S0), f64, rank-guarded."""
    S1 = 0.5 * (S1 + S1.T)
    S0 = 0.5 * (S0 + S0.T)
    d = np.sqrt(np.clip(np.diag(S0), 0, None))
    d = np.where(d > 0, d, 1.0)
    dn = 1.0 / d
    S0n = S0 * dn[:, None] * dn[None, :]
    S1n = S1 * dn[:, None] * dn[None, :]
    w0, v0 = np.linalg.eigh(S0n)
    keep = w0 > (w0.max() * CLIP_TH)
    v = v0[:, keep] / np.sqrt(w0[keep])[None, :]
    m = v.T @ S1n @ v
    m = 0.5 * (m + m.T)
    ev = np.linalg.eigvalsh(m)
    ev = np.clip(ev, 0.0, None)
    return np.sort(ev)[::-1][:k]


def _host_solve(results, k, c_scale):
    b = B_BLOCK
    nlev = Q_APPS + 1
    nblk = CHAINS * nlev
    P64 = np.zeros((nblk * b, nblk * b), dtype=np.float64)
    for r in results:
        p = r["p_out"].astype(np.float64)
        for a in range(nblk):
            for bb in range(a, nblk):
                blk = p[a * b:(a + 1) * b, bb * b:(bb + 1) * b]
                P64[a * b:(a + 1) * b, bb * b:(bb + 1) * b] += blk
                if bb != a:
                    P64[bb * b:(bb + 1) * b, a * b:(a + 1) * b] += blk.T
    bas = [c * nlev + t for c in range(CHAINS) for t in range(Q_APPS)]
    rows = np.concatenate([np.arange(a * b, (a + 1) * b) for a in bas])
    cols = np.concatenate([np.arange((a + 1) * b, (a + 2) * b) for a in bas])
    S0 = P64[np.ix_(rows, rows)]
    S1 = P64[np.ix_(rows, cols)]
    thetas = _ritz_topk(S1, S0, k)
    return float(np.sum(c_scale * thetas))


def _make_inputs(x_np, c_scale):
    bfd = ml_dtypes.bfloat16
    mloc = M_ROWS // N_CORES
    nloc = N_DIM // N_CORES
    xs = (x_np.astype(np.float64) / np.sqrt(c_scale)).astype(np.float32)
    xb = xs.astype(bfd)
    rng = np.random.default_rng(12345)
    omegas = [
        np.ascontiguousarray(
            rng.standard_normal((N_DIM, B_BLOCK)).astype(np.float32).astype(bfd)
        )
        for _ in range(CHAINS)
    ]
    in_maps = []
    for r in range(N_CORES):
        m = {
            "xrT": np.ascontiguousarray(xb[r * mloc:(r + 1) * mloc, :].T),
            "xc": np.ascontiguousarray(xb[:, r * nloc:(r + 1) * nloc]),
        }
        for c in range(CHAINS):
            m[f"omega{c}"] = omegas[c]
            m[f"omsl{c}"] = np.ascontiguousarray(
                omegas[c][r * nloc:(r + 1) * nloc, :]
            )
        in_maps.append(m)
    return in_maps


def _host_fallback(x_np, k_int):
    """Correct-but-slow host path, used only if the device result is bad."""
    import scipy.linalg

    g = x_np.astype(np.float64).T @ x_np.astype(np.float64)
    n = g.shape[0]
    ev = scipy.linalg.eigh(g, eigvals_only=True, subset_by_index=[n - k_int, n - 1])
    return float(np.sum(ev))


def kernel(x, k):
    from concourse.bass_utils import run_bass_kernel_spmd

    x_np = np.asarray(x, dtype=np.float32)
    k_int = int(np.asarray(k))
    if k_int <= 0:
        return np.asarray(0.0, dtype=np.float32)

    try:
        v = x_np.ravel()
        fro2 = float(np.dot(v, v))
        c_scale = 3.0 * fro2 / N_DIM
        cfg = (M_ROWS, N_DIM, B_BLOCK, Q_APPS, N_CORES, CHAINS)
        nc = _get_nc(cfg)
        in_maps = _make_inputs(x_np, c_scale)
        res = run_bass_kernel_spmd(nc, in_maps, core_ids=list(range(N_CORES)))
        val = _host_solve(res.results, k_int, c_scale)
        if not np.isfinite(val) or val <= 0:
            raise FloatingPointError(f"bad device result {val}")
    except Exception:
        val = _host_fallback(x_np, k_int)
    return np.asarray(val, dtype=np.float32)


# revision 4
# speedup vs baseline: 2.8254x; 1.1362x over previous
"""Distributed BatchSpectralLoss kernel for Trainium2 (8 NeuronCores).

Computes sum of top-k squared singular values of x (= top-k eigenvalues of
the Gram matrix G = x^T x) for x of shape (8192, 4096), k small (k=1).

Algorithm — implicit block Krylov on x (G is never formed):
  Host: scale x by 1/sqrt(C) with C = 3*||x||_F^2/N so lamhat_1 = O(1) in
  bf16, and draw `chains` random start blocks Omega [4096, b].
  Device, per core r (bf16 matmuls, fp32 PSUM):
    SBUF-resident x slices: xrT = x[rows_r,:]^T (lhsT for U = x_r @ Y) and
    xc = x[:, cols_r] (lhsT for the Y-update), rows_r = 1024 rows,
    cols_r = 512 cols.  All DRAM layouts are pi-major ([128, ...] with the
    partition index outermost) so DMA lines are >=1 KB contiguous.
    Per application t (q per chain, chains interleaved so one chain's
    matmuls hide the other's collectives):
      U_r = x[rows_r,:] @ Y_t          [1024, b]   (local rows)
      AllGather U slices -> U_full     [8192, b]
      Y_{t+1}[cols_r] = xc^T @ U_full  [512, b]    (local-complete, no AR)
      AllGather Y slices -> Y_full (skipped for the last level)
    (AllGather concatenates per-rank pi-major blocks; since rank blocks are
    contiguous multiples of 128 in k, tile index r*mo+pi keeps the same
    global-k ordering the lhsT tiling uses.)
    Basis Gram over the core's 512 rows: P[a,bb] = Y_a^T Y_bb for all
    stored levels (upper triangle, emitted as levels complete so the
    scheduler can fill collective-wait gaps); host sums partials.
  Host: S0 = P[basis, basis], S1 = P[basis, basis+1] (since
  Y_{t+1} = Ghat Y_t, so Y_i^T Ghat Y_j = Y_i^T Y_{j+1}); rank-guarded
  generalized Ritz values theta of (S1, S0); lambda = C * theta;
  answer = sum of top k.
"""

import numpy as np
import ml_dtypes

N_CORES = 8
M_ROWS = 8192
N_DIM = 4096
B_BLOCK = 128
Q_APPS = 5
CHAINS = 2
CLIP_TH = 1e-5
XCHUNKS = 4

_NC_CACHE: dict = {}


def _build_nc(m_rows, n_dim, b, q, n_cores, chains, enable_asserts=False):
    import concourse.mybir as mybir
    import concourse.tile as tile
    from concourse import bacc
    from contextlib import ExitStack

    P = 128
    mloc = m_rows // n_cores   # 1024 rows of x per core
    nloc = n_dim // n_cores    # 512 cols of x per core
    ko_u = n_dim // P          # 32 k-tiles for U-matmul
    ko_y = m_rows // P         # 64 k-tiles for Y-matmul
    mo_u = mloc // P           # 8 output tiles of U slice
    mo_y = nloc // P           # 4 output tiles of Y slice
    nlev = q + 1               # stored levels 0..q per chain
    nblk = chains * nlev
    nch = XCHUNKS
    kcu = ko_u // nch          # 8 k-tiles per xrT chunk
    kcy = ko_y // nch          # 16 k-tiles per xc chunk
    bf = mybir.dt.bfloat16
    f32 = mybir.dt.float32

    nc = bacc.Bacc(
        "TRN2",
        target_bir_lowering=False,
        debug=False,
        enable_asserts=enable_asserts,
        num_devices=n_cores,
    )

    # pi-major inputs: [...][pi, ko, m] = x-ish[ko*128 + pi, m]
    xrl = nc.dram_tensor("xrl", [P, ko_u, mloc], bf, kind="ExternalInput")
    xcl = nc.dram_tensor("xcl", [P, ko_y, nloc], bf, kind="ExternalInput")
    omega_l = [
        nc.dram_tensor(f"omega{c}", [P, n_cores, mo_y * b], bf, kind="ExternalInput")
        for c in range(chains)
    ]
    omsl = [
        nc.dram_tensor(f"omsl{c}", [P, mo_y, b], bf, kind="ExternalInput")
        for c in range(chains)
    ]
    p_out = nc.dram_tensor("p_out", [nblk * b, nblk * b], f32, kind="ExternalOutput")

    u_sl_d = [[nc.dram_tensor(f"usl_{c}_{t}", [P, mo_u, b], bf) for t in range(q)]
              for c in range(chains)]
    u_fl_d = [[nc.dram_tensor(f"ufl_{c}_{t}", [n_cores * P, mo_u * b], bf,
                              addr_space="Shared")
               for t in range(q)] for c in range(chains)]
    y_sl_d = [[nc.dram_tensor(f"ysl_{c}_{t}", [P, mo_y, b], bf) for t in range(q)]
              for c in range(chains)]
    y_fl_d = [[nc.dram_tensor(f"yfl_{c}_{t}", [n_cores * P, mo_y * b], bf,
                              addr_space="Shared")
               for t in range(q)] for c in range(chains)]

    rg = [list(range(n_cores))]

    def ag(inp, outp):
        nc.gpsimd.collective_compute(
            "AllGather",
            mybir.AluOpType.bypass,
            replica_groups=rg,
            ins=[inp.ap().opt()],
            outs=[outp.ap().opt()],
        )

    with tile.TileContext(nc) as tc, ExitStack() as ctx:
        xpool = ctx.enter_context(tc.tile_pool(name="xin", bufs=1))
        ypool = ctx.enter_context(tc.tile_pool(name="yfull", bufs=1))
        upool = ctx.enter_context(tc.tile_pool(name="ufull", bufs=1))
        slpool = ctx.enter_context(tc.tile_pool(name="slices", bufs=1))
        opool = ctx.enter_context(tc.tile_pool(name="evict", bufs=1))
        ppool = ctx.enter_context(tc.tile_pool(name="pout", bufs=3))
        # PSUM: 8 banks = chains*3 (application phase) + 2 (P-forms)
        pspool = ctx.enter_context(tc.tile_pool(name="ps", bufs=3, space="PSUM"))
        pspool2 = ctx.enter_context(tc.tile_pool(name="psp", bufs=2, space="PSUM"))

        # small start blocks first, then chunked x loads (compute can begin
        # as soon as the first chunk + omega land)
        ysl = {}
        ycur = {}
        for c in range(chains):
            yf = ypool.tile([P, n_cores, mo_y * b], bf, tag=f"yf{c}")
            nc.sync.dma_start(yf[:], omega_l[c].ap())
            ycur[c] = yf
            s = slpool.tile([P, mo_y, b], bf, tag=f"ysl{c}_0")
            nc.sync.dma_start(s[:], omsl[c].ap())
            ysl[(c, 0)] = s

        xr_ch = []
        xc_ch = []
        for i in range(nch):
            t_ = xpool.tile([P, kcu, mloc], bf, tag=f"xr{i}")
            nc.sync.dma_start(t_[:], xrl.ap()[:, i * kcu:(i + 1) * kcu, :])
            xr_ch.append(t_)
        for i in range(nch):
            t_ = xpool.tile([P, kcy, nloc], bf, tag=f"xc{i}")
            nc.sync.dma_start(t_[:], xcl.ap()[:, i * kcy:(i + 1) * kcy, :])
            xc_ch.append(t_)

        def rhs_u(tile_, ko):   # U-full rhs for global k-tile ko (= r*mo_u + mo)
            return tile_[:, ko // mo_u, (ko % mo_u) * b:(ko % mo_u + 1) * b]

        def rhs_y(tile_, ko):   # Y-full rhs for global k-tile ko (= r*mo_y + mo)
            return tile_[:, ko // mo_y, (ko % mo_y) * b:(ko % mo_y + 1) * b]

        stored = []  # block indices with slices available, in creation order

        def emit_p(z):
            for w in stored + [z]:
                a, bb = (w, z) if w < z else (z, w)
                ps = pspool2.tile([b, b], f32, tag="psp")
                ta = ysl[blocks[a]]
                tb = ysl[blocks[bb]]
                for ko in range(mo_y):
                    nc.tensor.matmul(
                        ps[:], ta[:, ko, :], tb[:, ko, :],
                        start=(ko == 0), stop=(ko == mo_y - 1),
                    )
                ob = ppool.tile([b, b], f32, tag="ob")
                nc.vector.tensor_copy(ob[:], ps[:])
                nc.sync.dma_start(
                    p_out.ap()[a * b:(a + 1) * b, bb * b:(bb + 1) * b], ob[:]
                )
            stored.append(z)

        blocks = [(c, t) for c in range(chains) for t in range(nlev)]
        bidx = {blk: i for i, blk in enumerate(blocks)}

        for c in range(chains):
            emit_p(bidx[(c, 0)])

        for t in range(q):
            for c in range(chains):
                # U slice = x[rows_r, :] @ Y_t
                usb = opool.tile([P, mo_u, b], bf, tag=f"u{c}")
                for mo in range(mo_u):
                    ps = pspool.tile([P, b], f32, tag=f"ps{c}")
                    for ko in range(ko_u):
                        nc.tensor.matmul(
                            ps[:],
                            xr_ch[ko // kcu][:, ko % kcu, mo * P:(mo + 1) * P],
                            rhs_y(ycur[c], ko),
                            start=(ko == 0),
                            stop=(ko == ko_u - 1),
                        )
                    nc.vector.tensor_copy(usb[:, mo, :], ps[:])
                nc.sync.dma_start(u_sl_d[c][t].ap(), usb[:])
                ag(u_sl_d[c][t], u_fl_d[c][t])
                ufs = upool.tile([P, n_cores, mo_u * b], bf, tag=f"uf{c}")
                nc.sync.dma_start(
                    ufs[:], u_fl_d[c][t].ap().rearrange("(r pi) w -> pi r w", pi=P)
                )
                # Y_{t+1} slice = x[:, cols_r]^T @ U_full (local-complete)
                ss = slpool.tile([P, mo_y, b], bf, tag=f"ysl{c}_{t + 1}")
                for mo in range(mo_y):
                    ps = pspool.tile([P, b], f32, tag=f"ps{c}")
                    for ko in range(ko_y):
                        nc.tensor.matmul(
                            ps[:],
                            xc_ch[ko // kcy][:, ko % kcy, mo * P:(mo + 1) * P],
                            rhs_u(ufs, ko),
                            start=(ko == 0),
                            stop=(ko == ko_y - 1),
                        )
                    nc.vector.tensor_copy(ss[:, mo, :], ps[:])
                ysl[(c, t + 1)] = ss
                if t + 1 < q:
                    nc.sync.dma_start(y_sl_d[c][t + 1].ap(), ss[:])
                    ag(y_sl_d[c][t + 1], y_fl_d[c][t + 1])
                    yf = ypool.tile([P, n_cores, mo_y * b], bf, tag=f"yf{c}")
                    nc.sync.dma_start(
                        yf[:],
                        y_fl_d[c][t + 1].ap().rearrange("(r pi) w -> pi r w", pi=P),
                    )
                    ycur[c] = yf
                emit_p(bidx[(c, t + 1)])

    nc.compile()
    return nc


def _get_nc(cfg):
    if cfg not in _NC_CACHE:
        _NC_CACHE[cfg] = _build_nc(*cfg)
    return _NC_CACHE[cfg]


def _ritz_topk(S1, S0, k):
    """Top-k generalized eigenvalues of (S1, S0), f64, rank-guarded."""
    S1 = 0.5 * (S1 + S1.T)
    S0 = 0.5 * (S0 + S0.T)
    d = np.sqrt(np.clip(np.diag(S0), 0, None))
    d = np.where(d > 0, d, 1.0)
    dn = 1.0 / d
    S0n = S0 * dn[:, None] * dn[None, :]
    S1n = S1 * dn[:, None] * dn[None, :]
    w0, v0 = np.linalg.eigh(S0n)
    keep = w0 > (w0.max() * CLIP_TH)
    v = v0[:, keep] / np.sqrt(w0[keep])[None, :]
    m = v.T @ S1n @ v
    m = 0.5 * (m + m.T)
    ev = np.linalg.eigvalsh(m)
    ev = np.clip(ev, 0.0, None)
    return np.sort(ev)[::-1][:k]


def _host_solve(results, k, c_scale):
    b = B_BLOCK
    nlev = Q_APPS + 1
    nblk = CHAINS * nlev
    P64 = np.zeros((nblk * b, nblk * b), dtype=np.float64)
    for r in results:
        p = r["p_out"].astype(np.float64)
        for a in range(nblk):
            for bb in range(a, nblk):
                blk = p[a * b:(a + 1) * b, bb * b:(bb + 1) * b]
                P64[a * b:(a + 1) * b, bb * b:(bb + 1) * b] += blk
                if bb != a:
                    P64[bb * b:(bb + 1) * b, a * b:(a + 1) * b] += blk.T
    bas = [c * nlev + t for c in range(CHAINS) for t in range(Q_APPS)]
    rows = np.concatenate([np.arange(a * b, (a + 1) * b) for a in bas])
    cols = np.concatenate([np.arange((a + 1) * b, (a + 2) * b) for a in bas])
    S0 = P64[np.ix_(rows, rows)]
    S1 = P64[np.ix_(rows, cols)]
    thetas = _ritz_topk(S1, S0, k)
    return float(np.sum(c_scale * thetas))


def _pi_major(a):
    """[K, m] -> [128, K//128, m] with out[pi, ko, m] = a[ko*128 + pi, m]."""
    K, m = a.shape
    return np.ascontiguousarray(a.reshape(K // 128, 128, m).transpose(1, 0, 2))


def _make_inputs(x_np, c_scale):
    bfd = ml_dtypes.bfloat16
    mloc = M_ROWS // N_CORES
    nloc = N_DIM // N_CORES
    b = B_BLOCK
    xs = (x_np.astype(np.float64) / np.sqrt(c_scale)).astype(np.float32)
    xb = xs.astype(bfd)
    rng = np.random.default_rng(12345)
    omegas = [
        rng.standard_normal((N_DIM, b)).astype(np.float32).astype(bfd)
        for _ in range(CHAINS)
    ]
    # omega in Y-full layout: [pi, r, mo*b + j] = omega[512 r + 128 mo + pi, j]
    om_l = [
        np.ascontiguousarray(
            om.reshape(N_CORES, nloc // 128, 128, b).transpose(2, 0, 1, 3)
            .reshape(128, N_CORES, (nloc // 128) * b)
        )
        for om in omegas
    ]
    in_maps = []
    for r in range(N_CORES):
        m = {
            "xrl": _pi_major(np.ascontiguousarray(xb[r * mloc:(r + 1) * mloc, :].T)),
            "xcl": _pi_major(xb[:, r * nloc:(r + 1) * nloc]),
        }
        for c in range(CHAINS):
            m[f"omega{c}"] = om_l[c]
            m[f"omsl{c}"] = _pi_major(omegas[c][r * nloc:(r + 1) * nloc, :])
        in_maps.append(m)
    return in_maps


def _host_fallback(x_np, k_int):
    """Correct-but-slow host path, used only if the device result is bad."""
    import scipy.linalg

    g = x_np.astype(np.float64).T @ x_np.astype(np.float64)
    n = g.shape[0]
    ev = scipy.linalg.eigh(g, eigvals_only=True, subset_by_index=[n - k_int, n - 1])
    return float(np.sum(ev))


def kernel(x, k):
    from concourse.bass_utils import run_bass_kernel_spmd

    x_np = np.asarray(x, dtype=np.float32)
    k_int = int(np.asarray(k))
    if k_int <= 0:
        return np.asarray(0.0, dtype=np.float32)

    try:
        v = x_np.ravel()
        fro2 = float(np.dot(v, v))
        c_scale = 3.0 * fro2 / N_DIM
        cfg = (M_ROWS, N_DIM, B_BLOCK, Q_APPS, N_CORES, CHAINS)
        nc = _get_nc(cfg)
        in_maps = _make_inputs(x_np, c_scale)
        res = run_bass_kernel_spmd(nc, in_maps, core_ids=list(range(N_CORES)))
        val = _host_solve(res.results, k_int, c_scale)
        if not np.isfinite(val) or val <= 0:
            raise FloatingPointError(f"bad device result {val}")
    except Exception:
        val = _host_fallback(x_np, k_int)
    return np.asarray(val, dtype=np.float32)
